# revision 2
# baseline (speedup 1.0000x reference)
"""NNUE MoE-routing forward pass on 8 Trainium2 NeuronCores (Bass/Tile).

Sharding: data-parallel over the batch. Each core processes B/8 = 512
samples (4 chunks of 128 samples = SBUF partition dim); the 186MB feature
table and the small layer-stack params are replicated per core.

Per chunk of 128 samples:
  - 64 indirect-DMA row gathers (white+black active features) from the
    HBM-resident table, accumulated on DVE with scalar_tensor_tensor
    chains: acc = gathered * value + acc (ft_bias seeds the chain).
  - Perspective select via us in {0,1}, clip, pairwise feature crossing,
    router logits; the reference's straight-through gumbel-softmax one-hot
    is numerically exactly one_hot(argmax(logits + gumbel)), computed as
    an is_equal mask against the row max.
  - All-bucket layer stack as PE matmuls (biases folded in as an extra
    ones-row contraction), bucket selection via the one-hot mask, psqt
    residual.
"""

import sys
for _p in ('/opt/pypackages', '/opt/trn_rl_repo'):
    if _p not in sys.path:
        sys.path.insert(0, _p)

import numpy as np

import concourse.bass as bass
import concourse.mybir as mybir
import concourse.tile as tile
from concourse.bass import IndirectOffsetOnAxis
from concourse.bass_utils import run_bass_kernel_spmd
from concourse.masks import make_identity

F32 = mybir.dt.float32
I32 = mybir.dt.int32
Alu = mybir.AluOpType

L1, L2, L3 = 1024, 15, 32
NB = 8
NRF = 16
MAX_FT = 1.0
L0C = np.float32(127.0 / 128.0)
NF = 45056
O = L1 + NB  # 1032
M = 32
B = 4096
N_CORES = 8
B_CORE = B // N_CORES  # 512
P = 128
NCHUNK = B_CORE // P  # 4
H = L1 // 2  # 512


def _split_excess_waits(nc, max_waits=1):
    """This neuronxcc's CoreV3 codegen rejects instructions with more than
    one sync wait; hoist excess waits onto preceding NoOp carriers."""
    for f in nc.m.functions:
        for blk in f.blocks:
            out = []
            for ins in blk.instructions:
                si = ins.sync_info
                if si is not None and len(si.on_wait) > max_waits:
                    waits = list(si.on_wait)
                    extra, keep = waits[:-max_waits], waits[-max_waits:]
                    for i in range(0, len(extra), max_waits):
                        chunk = extra[i:i + max_waits]
                        nop = mybir.InstNoOp(
                            name=f"{ins.name}_wsplit{i}", ins=[], outs=[])
                        nop.engine = ins.engine
                        nop.sync_info = mybir.SyncInfo(
                            on_wait=chunk, on_update=[])
                        out.append(nop)
                    ins.sync_info = mybir.SyncInfo(
                        on_wait=keep, on_update=list(si.on_update))
                out.append(ins)
            blk.instructions = out


def _build(nc):
    wi = nc.declare_dram_parameter("wi", [NCHUNK, P, M], I32, isOutput=False)
    bi = nc.declare_dram_parameter("bi", [NCHUNK, P, M], I32, isOutput=False)
    wv = nc.declare_dram_parameter("wv", [NCHUNK, P, M], F32, isOutput=False)
    bv = nc.declare_dram_parameter("bv", [NCHUNK, P, M], F32, isOutput=False)
    us = nc.declare_dram_parameter("us", [NCHUNK, P], F32, isOutput=False)
    gmb = nc.declare_dram_parameter("gmb", [NCHUNK, P, NB], F32, isOutput=False)
    ftw = nc.declare_dram_parameter("ftw", [NF, O], F32, isOutput=False)
    ftb = nc.declare_dram_parameter("ftb", [1, O], F32, isOutput=False)
    rwp = nc.declare_dram_parameter("rw", [NB, 2 * NRF], F32, isOutput=False)
    rbp = nc.declare_dram_parameter("rb", [1, NB], F32, isOutput=False)
    rls = nc.declare_dram_parameter("rls", [1, 1], F32, isOutput=False)
    l1w = nc.declare_dram_parameter("l1w", [(L2 + 1) * NB, L1], F32, isOutput=False)
    l1b = nc.declare_dram_parameter("l1b", [1, (L2 + 1) * NB], F32, isOutput=False)
    l2w = nc.declare_dram_parameter("l2w", [L3 * NB, 2 * L2], F32, isOutput=False)
    l2b = nc.declare_dram_parameter("l2b", [1, L3 * NB], F32, isOutput=False)
    ow = nc.declare_dram_parameter("ow", [NB, L3], F32, isOutput=False)
    ob = nc.declare_dram_parameter("ob", [1, NB], F32, isOutput=False)
    out = nc.declare_dram_parameter("out", [NCHUNK, P], F32, isOutput=True)

    K1 = (L2 + 1) * NB  # 128
    K2 = L3 * NB        # 256

    from contextlib import ExitStack
    with tile.TileContext(nc) as tc, ExitStack() as ctx:
        cp = ctx.enter_context(tc.tile_pool(name="const", bufs=1))
        gp = ctx.enter_context(tc.tile_pool(name="gather", bufs=8))
        wp = ctx.enter_context(tc.tile_pool(name="work", bufs=2))
        pp_big = ctx.enter_context(tc.tile_pool(name="ppbig", bufs=1, space="PSUM"))
        pp_mm = ctx.enter_context(tc.tile_pool(name="ppmm", bufs=2, space="PSUM"))
        pp_l2 = ctx.enter_context(tc.tile_pool(name="ppl2", bufs=1, space="PSUM"))
        pp_tr = ctx.enter_context(tc.tile_pool(name="pptr", bufs=2, space="PSUM"))

        # ---- one-time setup ----------------------------------------------
        ident = cp.tile([P, P], F32)
        make_identity(nc, ident[:])
        ones_row = cp.tile([1, P], F32)
        nc.vector.memset(ones_row[:], 1.0)

        ftb_sb = cp.tile([1, O], F32)
        nc.scalar.dma_start(out=ftb_sb[:], in_=ftb[:])
        rw_sb = cp.tile([NB, 2 * NRF], F32)
        nc.scalar.dma_start(out=rw_sb[:], in_=rwp[:])
        rb_sb = cp.tile([1, NB], F32)
        nc.scalar.dma_start(out=rb_sb[:], in_=rbp[:])
        rls_sb = cp.tile([1, 1], F32)
        nc.scalar.dma_start(out=rls_sb[:], in_=rls[:])
        l1w_sb = cp.tile([K1, L1], F32)
        nc.scalar.dma_start(out=l1w_sb[:], in_=l1w[:])
        l1b_sb = cp.tile([1, K1], F32)
        nc.scalar.dma_start(out=l1b_sb[:], in_=l1b[:])
        l2w_sb = cp.tile([P, 2 * (2 * L2)], F32)
        nc.scalar.dma_start(out=l2w_sb[:, 0:2 * L2], in_=l2w[0:P, :])
        nc.scalar.dma_start(out=l2w_sb[:, 2 * L2:4 * L2], in_=l2w[P:K2, :])
        l2b_sb = cp.tile([1, K2], F32)
        nc.scalar.dma_start(out=l2b_sb[:], in_=l2b[:])
        ow_sb = cp.tile([NB, L3], F32)
        nc.scalar.dma_start(out=ow_sb[:], in_=ow[:])
        ob_sb = cp.tile([1, NB], F32)
        nc.scalar.dma_start(out=ob_sb[:], in_=ob[:])

        idx_w = cp.tile([P, NCHUNK * M], I32)
        idx_b = cp.tile([P, NCHUNK * M], I32)
        val_w = cp.tile([P, NCHUNK * M], F32)
        val_b = cp.tile([P, NCHUNK * M], F32)
        us_sb = cp.tile([P, NCHUNK], F32)
        gmb_sb = cp.tile([P, NCHUNK * NB], F32)
        for c in range(NCHUNK):
            nc.scalar.dma_start(out=idx_w[:, c * M:(c + 1) * M], in_=wi[c])
            nc.scalar.dma_start(out=idx_b[:, c * M:(c + 1) * M], in_=bi[c])
            nc.scalar.dma_start(out=val_w[:, c * M:(c + 1) * M], in_=wv[c])
            nc.scalar.dma_start(out=val_b[:, c * M:(c + 1) * M], in_=bv[c])
            nc.scalar.dma_start(out=us_sb[:, c:c + 1], in_=us[c])
            nc.scalar.dma_start(out=gmb_sb[:, c * NB:(c + 1) * NB], in_=gmb[c])

        # replicate ft_bias across partitions via ones-row matmul
        bias_full = cp.tile([P, O], F32)
        for lo, hi in ((0, 512), (512, 1024), (1024, O)):
            pb = pp_mm.tile([P, 512], F32, tag="mmlx")
            w_ = hi - lo
            nc.tensor.matmul(out=pb[:, 0:w_], lhsT=ones_row[:],
                             rhs=ftb_sb[:, lo:hi], start=True, stop=True)
            nc.vector.tensor_copy(out=bias_full[:, lo:hi], in_=pb[:, 0:w_])

        pls = pp_mm.tile([P, 512], F32, tag="mmlx")
        nc.tensor.matmul(out=pls[:, 0:1], lhsT=ones_row[:], rhs=rls_sb[:],
                         start=True, stop=True)
        ls_col = cp.tile([P, 1], F32)
        nc.vector.tensor_copy(out=ls_col[:], in_=pls[:, 0:1])

        # l1wT tiles (transposed, pre-scaled by the l0 correction factor)
        l1wT = cp.tile([P, L1], F32)
        for t in range(8):
            pt = pp_tr.tile([P, P], F32, tag="tr")
            nc.tensor.transpose(out=pt[:], in_=l1w_sb[:, t * P:(t + 1) * P],
                                identity=ident[:])
            nc.vector.tensor_scalar(out=l1wT[:, t * P:(t + 1) * P], in0=pt[:],
                                    scalar1=float(L0C), scalar2=None, op0=Alu.mult)

        l2wT = cp.tile([2 * L2, K2], F32)
        for h_ in range(2):
            pt = pp_tr.tile([P, P], F32, tag="tr")
            nc.tensor.transpose(out=pt[0:2 * L2, :],
                                in_=l2w_sb[:, h_ * (2 * L2):(h_ + 1) * (2 * L2)],
                                identity=ident[:])
            nc.vector.tensor_copy(out=l2wT[:, h_ * P:(h_ + 1) * P],
                                  in_=pt[0:2 * L2, :])

        rwT = cp.tile([2 * NRF, NB], F32)
        pt = pp_tr.tile([P, P], F32, tag="tr")
        nc.tensor.transpose(out=pt[0:2 * NRF, 0:NB], in_=rw_sb[:],
                            identity=ident[0:NB, 0:NB])
        nc.vector.tensor_copy(out=rwT[:], in_=pt[0:2 * NRF, 0:NB])
        owT = cp.tile([L3, NB], F32)
        pt = pp_tr.tile([P, P], F32, tag="tr")
        nc.tensor.transpose(out=pt[0:L3, 0:NB], in_=ow_sb[:],
                            identity=ident[0:NB, 0:NB])
        nc.vector.tensor_copy(out=owT[:], in_=pt[0:L3, 0:NB])

        # ---- per-chunk pipeline ------------------------------------------
        for c in range(NCHUNK):
            acc_w = wp.tile([P, O], F32, tag="acc_w")
            acc_b = wp.tile([P, O], F32, tag="acc_b")
            for m in range(M):
                col = c * M + m
                g1 = gp.tile([P, O], F32, tag="g")
                nc.gpsimd.indirect_dma_start(
                    out=g1[:], out_offset=None, in_=ftw[:],
                    in_offset=IndirectOffsetOnAxis(ap=idx_w[:, col:col + 1], axis=0))
                nc.vector.scalar_tensor_tensor(
                    out=acc_w[:], in0=g1[:], scalar=val_w[:, col:col + 1],
                    in1=(bias_full[:] if m == 0 else acc_w[:]),
                    op0=Alu.mult, op1=Alu.add)
                g2 = gp.tile([P, O], F32, tag="g")
                nc.gpsimd.indirect_dma_start(
                    out=g2[:], out_offset=None, in_=ftw[:],
                    in_offset=IndirectOffsetOnAxis(ap=idx_b[:, col:col + 1], axis=0))
                nc.vector.scalar_tensor_tensor(
                    out=acc_b[:], in0=g2[:], scalar=val_b[:, col:col + 1],
                    in1=(bias_full[:] if m == 0 else acc_b[:]),
                    op0=Alu.mult, op1=Alu.add)

            usc = us_sb[:, c:c + 1]
            neg_us = wp.tile([P, 1], F32, tag="neg_us")
            nc.vector.tensor_scalar(out=neg_us[:], in0=usc, scalar1=-1.0,
                                    scalar2=None, op0=Alu.mult)
            um = wp.tile([P, 1], F32, tag="um")
            nc.vector.tensor_scalar(out=um[:], in0=usc, scalar1=0.5,
                                    scalar2=None, op0=Alu.subtract)

            # perspective select: l0a = us*w + (1-us)*b ; l0b = us*b + (1-us)*w
            dt_ = wp.tile([P, L1], F32, tag="dt")
            nc.vector.scalar_tensor_tensor(
                out=dt_[:], in0=acc_w[:, 0:L1], scalar=1.0, in1=acc_b[:, 0:L1],
                op0=Alu.mult, op1=Alu.subtract)
            l0a = wp.tile([P, L1], F32, tag="l0a")
            nc.vector.scalar_tensor_tensor(
                out=l0a[:], in0=dt_[:], scalar=usc, in1=acc_b[:, 0:L1],
                op0=Alu.mult, op1=Alu.add)
            l0b = wp.tile([P, L1], F32, tag="l0b")
            nc.vector.scalar_tensor_tensor(
                out=l0b[:], in0=dt_[:], scalar=neg_us[:], in1=acc_w[:, 0:L1],
                op0=Alu.mult, op1=Alu.add)
            nc.vector.tensor_scalar(out=l0a[:], in0=l0a[:], scalar1=0.0,
                                    scalar2=MAX_FT, op0=Alu.max, op1=Alu.min)
            nc.vector.tensor_scalar(out=l0b[:], in0=l0b[:], scalar1=0.0,
                                    scalar2=MAX_FT, op0=Alu.max, op1=Alu.min)

            # pairwise crossing (unscaled; L0C folded into l1wT)
            pq = wp.tile([P, L1], F32, tag="pq")
            nc.vector.scalar_tensor_tensor(
                out=pq[:, 0:H], in0=l0a[:, 0:H], scalar=1.0, in1=l0a[:, H:L1],
                op0=Alu.mult, op1=Alu.mult)
            nc.vector.scalar_tensor_tensor(
                out=pq[:, H:L1], in0=l0b[:, 0:H], scalar=1.0, in1=l0b[:, H:L1],
                op0=Alu.mult, op1=Alu.mult)

            # router features -> logits -> hard one-hot
            feats = wp.tile([P, 2 * NRF], F32, tag="feats")
            nc.vector.tensor_copy(out=feats[:, 0:NRF], in_=pq[:, H - NRF:H])
            nc.vector.tensor_copy(out=feats[:, NRF:2 * NRF], in_=pq[:, L1 - NRF:L1])
            ptf = pp_tr.tile([P, P], F32, tag="tr")
            nc.tensor.transpose(out=ptf[0:2 * NRF, :], in_=feats[:], identity=ident[:])
            featsT = wp.tile([2 * NRF, P], F32, tag="featsT")
            nc.vector.tensor_copy(out=featsT[:], in_=ptf[0:2 * NRF, :])
            pr = pp_mm.tile([P, 512], F32, tag="mmlx")
            nc.tensor.matmul(out=pr[:, 0:NB], lhsT=featsT[:], rhs=rwT[:],
                             start=True, stop=False)
            nc.tensor.matmul(out=pr[:, 0:NB], lhsT=ones_row[:], rhs=rb_sb[:],
                             start=False, stop=True)
            z = wp.tile([P, NB], F32, tag="z")
            nc.vector.scalar_tensor_tensor(
                out=z[:], in0=pr[:, 0:NB], scalar=ls_col[:],
                in1=gmb_sb[:, c * NB:(c + 1) * NB], op0=Alu.mult, op1=Alu.add)
            zmax = wp.tile([P, 1], F32, tag="zmax")
            nc.vector.tensor_reduce(out=zmax[:], in_=z[:],
                                    axis=mybir.AxisListType.X, op=Alu.max)
            rwoh = wp.tile([P, NB], F32, tag="rwoh")
            nc.vector.tensor_scalar(out=rwoh[:], in0=z[:], scalar1=zmax[:],
                                    scalar2=None, op0=Alu.is_equal)

            # l0_ transpose tiles
            ppq = pp_big.tile([P, L1], F32, tag="pqT")
            for t in range(8):
                nc.tensor.transpose(out=ppq[:, t * P:(t + 1) * P],
                                    in_=pq[:, t * P:(t + 1) * P], identity=ident[:])
            pqT = wp.tile([P, L1], F32, tag="pqT_sb")
            nc.vector.tensor_copy(out=pqT[:], in_=ppq[:])

            # l1 stack for all buckets: [128s, 128k]
            pl1 = pp_mm.tile([P, 512], F32, tag="mmlx")
            for t in range(8):
                nc.tensor.matmul(out=pl1[:, 0:K1], lhsT=pqT[:, t * P:(t + 1) * P],
                                 rhs=l1wT[:, t * P:(t + 1) * P],
                                 start=(t == 0), stop=False)
            nc.tensor.matmul(out=pl1[:, 0:K1], lhsT=ones_row[:], rhs=l1b_sb[:],
                             start=False, stop=True)

            l1c = wp.tile([P, L2 + 1], F32, tag="l1c")
            nc.vector.tensor_scalar(out=l1c[:], in0=pl1[:, 0:L2 + 1],
                                    scalar1=rwoh[:, 0:1], scalar2=None, op0=Alu.mult)
            for n in range(1, NB):
                nc.vector.scalar_tensor_tensor(
                    out=l1c[:], in0=pl1[:, n * (L2 + 1):(n + 1) * (L2 + 1)],
                    scalar=rwoh[:, n:n + 1], in1=l1c[:], op0=Alu.mult, op1=Alu.add)

            l1x = wp.tile([P, 2 * L2], F32, tag="l1x")
            nc.vector.tensor_scalar(out=l1x[:, L2:2 * L2], in0=l1c[:, 0:L2],
                                    scalar1=0.0, scalar2=1.0, op0=Alu.max, op1=Alu.min)
            nc.vector.scalar_tensor_tensor(
                out=l1x[:, 0:L2], in0=l1x[:, L2:2 * L2], scalar=float(L0C),
                in1=l1x[:, L2:2 * L2], op0=Alu.mult, op1=Alu.mult)

            ptx = pp_tr.tile([P, P], F32, tag="tr")
            nc.tensor.transpose(out=ptx[0:2 * L2, :], in_=l1x[:], identity=ident[:])
            l1xT = wp.tile([2 * L2, P], F32, tag="l1xT")
            nc.vector.tensor_copy(out=l1xT[:], in_=ptx[0:2 * L2, :])

            pl2 = pp_l2.tile([P, K2], F32, tag="l2")
            nc.tensor.matmul(out=pl2[:], lhsT=l1xT[:], rhs=l2wT[:],
                             start=True, stop=False)
            nc.tensor.matmul(out=pl2[:], lhsT=ones_row[:], rhs=l2b_sb[:],
                             start=False, stop=True)
            l2c = wp.tile([P, L3], F32, tag="l2c")
            nc.vector.tensor_scalar(out=l2c[:], in0=pl2[:, 0:L3],
                                    scalar1=rwoh[:, 0:1], scalar2=None, op0=Alu.mult)
            for n in range(1, NB):
                nc.vector.scalar_tensor_tensor(
                    out=l2c[:], in0=pl2[:, n * L3:(n + 1) * L3],
                    scalar=rwoh[:, n:n + 1], in1=l2c[:], op0=Alu.mult, op1=Alu.add)
            nc.vector.tensor_scalar(out=l2c[:], in0=l2c[:], scalar1=0.0,
                                    scalar2=1.0, op0=Alu.max, op1=Alu.min)

            ptc = pp_tr.tile([P, P], F32, tag="tr")
            nc.tensor.transpose(out=ptc[0:L3, :], in_=l2c[:], identity=ident[:])
            l2cT = wp.tile([L3, P], F32, tag="l2cT")
            nc.vector.tensor_copy(out=l2cT[:], in_=ptc[0:L3, :])

            pl3 = pp_mm.tile([P, 512], F32, tag="mmlx")
            nc.tensor.matmul(out=pl3[:, 0:NB], lhsT=l2cT[:], rhs=owT[:],
                             start=True, stop=False)
            nc.tensor.matmul(out=pl3[:, 0:NB], lhsT=ones_row[:], rhs=ob_sb[:],
                             start=False, stop=True)

            s8 = wp.tile([P, NB], F32, tag="s8")
            l3c = wp.tile([P, 1], F32, tag="l3c")
            nc.vector.scalar_tensor_tensor(
                out=s8[:], in0=pl3[:, 0:NB], scalar=1.0, in1=rwoh[:],
                op0=Alu.mult, op1=Alu.mult)
            nc.vector.tensor_reduce(out=l3c[:], in_=s8[:],
                                    axis=mybir.AxisListType.X, op=Alu.add)
            s8b = wp.tile([P, NB], F32, tag="s8b")
            wsel = wp.tile([P, 1], F32, tag="wsel")
            nc.vector.scalar_tensor_tensor(
                out=s8b[:], in0=acc_w[:, L1:O], scalar=1.0, in1=rwoh[:],
                op0=Alu.mult, op1=Alu.mult)
            nc.vector.tensor_reduce(out=wsel[:], in_=s8b[:],
                                    axis=mybir.AxisListType.X, op=Alu.add)
            s8c = wp.tile([P, NB], F32, tag="s8c")
            bsel = wp.tile([P, 1], F32, tag="bsel")
            nc.vector.scalar_tensor_tensor(
                out=s8c[:], in0=acc_b[:, L1:O], scalar=1.0, in1=rwoh[:],
                op0=Alu.mult, op1=Alu.mult)
            nc.vector.tensor_reduce(out=bsel[:], in_=s8c[:],
                                    axis=mybir.AxisListType.X, op=Alu.add)

            t0 = wp.tile([P, 1], F32, tag="t0")
            nc.vector.scalar_tensor_tensor(
                out=t0[:], in0=l1c[:, L2:L2 + 1], scalar=1.0, in1=l3c[:],
                op0=Alu.mult, op1=Alu.add)
            dsel = wp.tile([P, 1], F32, tag="dsel")
            nc.vector.scalar_tensor_tensor(
                out=dsel[:], in0=wsel[:], scalar=1.0, in1=bsel[:],
                op0=Alu.mult, op1=Alu.subtract)
            res = wp.tile([P, 1], F32, tag="res")
            nc.vector.scalar_tensor_tensor(
                out=res[:], in0=dsel[:], scalar=um[:], in1=t0[:],
                op0=Alu.mult, op1=Alu.add)
            nc.scalar.dma_start(out=out[c], in_=res[:])
    return nc


_NC_CACHE = None


def _get_nc():
    global _NC_CACHE
    if _NC_CACHE is None:
        nc = bass.Bass("TRN2", target_bir_lowering=False)
        _build(nc)
        _split_excess_waits(nc, max_waits=1)
        _NC_CACHE = nc
    return _NC_CACHE


def _make_in_maps(inputs):
    f32 = np.float32
    wi = np.ascontiguousarray(np.asarray(inputs["white_indices"]).astype(np.int32))
    bi = np.ascontiguousarray(np.asarray(inputs["black_indices"]).astype(np.int32))
    wv = np.ascontiguousarray(np.asarray(inputs["white_values"], dtype=f32))
    bv = np.ascontiguousarray(np.asarray(inputs["black_values"], dtype=f32))
    us = np.ascontiguousarray(np.asarray(inputs["us"], dtype=f32)).reshape(B)
    gmb = np.ascontiguousarray(np.asarray(inputs["gumbel_noise"], dtype=f32))
    shared = dict(
        ftw=np.ascontiguousarray(np.asarray(inputs["ft_weight"], dtype=f32)),
        ftb=np.asarray(inputs["ft_bias"], dtype=f32).reshape(1, O),
        rw=np.ascontiguousarray(np.asarray(inputs["router_w"], dtype=f32)),
        rb=np.asarray(inputs["router_b"], dtype=f32).reshape(1, NB),
        rls=np.asarray(inputs["router_ls"], dtype=f32).reshape(1, 1),
        l1w=np.ascontiguousarray(np.asarray(inputs["l1_w"], dtype=f32)),
        l1b=np.asarray(inputs["l1_b"], dtype=f32).reshape(1, (L2 + 1) * NB),
        l2w=np.ascontiguousarray(np.asarray(inputs["l2_w"], dtype=f32)),
        l2b=np.asarray(inputs["l2_b"], dtype=f32).reshape(1, L3 * NB),
        ow=np.ascontiguousarray(np.asarray(inputs["out_w"], dtype=f32)),
        ob=np.asarray(inputs["out_b"], dtype=f32).reshape(1, NB),
    )
    maps = []
    for core in range(N_CORES):
        lo, hi = core * B_CORE, (core + 1) * B_CORE
        maps.append(dict(
            shared,
            wi=wi[lo:hi].reshape(NCHUNK, P, M),
            bi=bi[lo:hi].reshape(NCHUNK, P, M),
            wv=wv[lo:hi].reshape(NCHUNK, P, M),
            bv=bv[lo:hi].reshape(NCHUNK, P, M),
            us=us[lo:hi].reshape(NCHUNK, P),
            gmb=gmb[lo:hi].reshape(NCHUNK, P, NB),
        ))
    return maps


def run(inputs, trace=False):
    nc = _get_nc()
    maps = _make_in_maps(inputs)
    return run_bass_kernel_spmd(nc, maps, list(range(N_CORES)), trace=trace)


def kernel(**inputs):
    r = run(inputs, trace=False)
    return np.concatenate(
        [r.results[i]["out"].reshape(B_CORE, 1) for i in range(N_CORES)],
        axis=0).astype(np.float32)
